# revision 26
# baseline (speedup 1.0000x reference)
"""Trainium2 Bass kernel for nn_ExHuneLSTM (bidirectional single-step LSTM scan).

Key observation: the forward-direction LSTM in the reference is dead code.
``x_hat = out1[:, -1:]`` is the last channel of ``concat(hf, hb)`` — i.e.
``hb[:, -1]`` — and the backward cell never reads ``hf/cf``.  Only the
backward-parameter cell chain affects the output:

  per step t:  hb1, cb1 = cell_b(u_t, hb, cb)        xh  = hb1[:, 511]
               hb2, cb2 = cell_b(u_t + xh, hb1, cb1) xhn = hb2[:, 511]
               out[t] = 1.5*u_t + 0.5*(xh + xhn)

Implementation notes (per core, batch sharded 8 ways, B_local=4):
  * P[t] = u_t @ w_ih.T + (b_ih + b_hh) precomputed for all t with one big
    GEMM pass, streamed back per step. Phase B's input projection is
    P[t] + xh * rowsum(w_ih): a rank-1 correction folded into the gate
    accumulation (applied last so the matmuls can start early).
  * Gate pre-activations in PSUM as (128, 512): partition 32*q + b
    (q = H-quarter, b = batch), free = [i|f|g|o] x 128.  Weights are
    pre-permuted on host to quarter-major gate-interleaved order; g-gate
    columns pre-scaled by 2 so tanh(g) = 2*sigmoid(2g)-1 comes out of the
    same fused sigmoid.
  * Recurrent matmul: stationary = hT column slices (128, 4) from a compact
    transposed state hT (128, 16) (col 4k+b = quarter k, batch b); moving =
    w_hh.T chunks (128, 512) bf16 through 4 PE column groups concurrently
    (tile_position=(0, 32*j)).
  * h transpose uses a (128, 16) selector as the transpose identity, so the
    PE streams 16 columns instead of 128 and the PSUM->SBUF copy is (128,16).
    xh is then just hT[127, 12:16] — no separate extraction transpose.
"""

import math
import os
from contextlib import ExitStack

import numpy as np
import ml_dtypes

import concourse.bass as bass
import concourse.mybir as mybir
import concourse.tile as tile
from concourse import bacc
from concourse.bass import ds
from concourse.bass_utils import run_bass_kernel_spmd

F32 = mybir.dt.float32
BF16 = mybir.dt.bfloat16
AF = mybir.ActivationFunctionType
OP = mybir.AluOpType

B, D, H = 32, 512, 512
G4 = 4 * H  # 2048 gate dim
N_CORES = 8
BL = B // N_CORES  # 4 batch rows per core

_BF = ml_dtypes.bfloat16


# ---------------------------------------------------------------- host prep --

def _perm_rows(x):
    """Permute gate-dim (4H) from gate-major (g,q,h) to quarter-major
    (q,g,h) along axis 0."""
    s = x.shape
    y = x.reshape(4, 4, 128, *s[1:])
    return y.transpose(1, 0, 2, *range(3, y.ndim)).reshape(*s)


def _prep_dir(w_ih, w_hh, b_ih, b_hh):
    """Returns (whh_img (128, 8192) bf16, wih_img (128, 8192) bf16,
    bias (1, 2048) f32, rowsum (1, 2048) bf16) with gate permutation and
    2x pre-scale on the g gate."""
    w_ih = np.asarray(w_ih, np.float32).copy()
    w_hh = np.asarray(w_hh, np.float32).copy()
    bias = (np.asarray(b_ih, np.float32) + np.asarray(b_hh, np.float32)).copy()
    rowsum = w_ih.sum(axis=1)
    # 2x scale on g gate (PyTorch order i,f,g,o -> rows 1024:1536)
    w_ih[1024:1536] *= 2.0
    w_hh[1024:1536] *= 2.0
    bias = bias.copy(); bias[1024:1536] *= 2.0
    rowsum = rowsum.copy(); rowsum[1024:1536] *= 2.0
    w_ih = _perm_rows(w_ih)
    w_hh = _perm_rows(w_hh)
    bias = _perm_rows(bias)
    rowsum = _perm_rows(rowsum)

    def img(w):  # (2048, 512) -> w.T (512, 2048) -> K-chunks side by side
        wt = w.T.astype(_BF)                       # (512, 2048)
        return np.ascontiguousarray(
            wt.reshape(4, 128, G4).transpose(1, 0, 2).reshape(128, 4 * G4))

    return (img(w_hh), img(w_ih),
            bias.reshape(1, G4).astype(np.float32),
            rowsum.reshape(1, G4).astype(_BF))


# ------------------------------------------------------------ device program --

def build_program(T, unroll=8, use_loop=True):
    nc = bacc.Bacc("TRN2", num_devices=N_CORES, debug=False)

    u_d = nc.dram_tensor("u", (BL, T, D), F32, kind="ExternalInput")
    whh_d = nc.dram_tensor("whh_b", (128, 4 * G4), BF16, kind="ExternalInput")
    wih_d = nc.dram_tensor("wih_b", (128, 4 * G4), BF16, kind="ExternalInput")
    bias_d = nc.dram_tensor("bias_b", (1, G4), F32, kind="ExternalInput")
    rs_d = nc.dram_tensor("rs_b", (1, G4), BF16, kind="ExternalInput")
    ident_d = nc.dram_tensor("ident", (128, 128), BF16, kind="ExternalInput")
    s16_d = nc.dram_tensor("s16", (16, 128), BF16, kind="ExternalInput")
    sel16_d = nc.dram_tensor("sel16", (128, 16), BF16, kind="ExternalInput")
    out_d = nc.dram_tensor("out", (BL, T, D), F32, kind="ExternalOutput")

    pb_d = nc.dram_tensor("pb_scratch", (T, 16, 512), BF16, kind="Internal")
    s_d = nc.dram_tensor("s_scratch", (BL * T,), F32, kind="Internal")

    ntok = BL * T
    nchunk = math.ceil(ntok / 128)

    with ExitStack() as ctx:
        tc = ctx.enter_context(tile.TileContext(nc))

        consts = ctx.enter_context(tc.tile_pool(name="consts", bufs=1))
        wpool = ctx.enter_context(tc.tile_pool(name="weights", bufs=1))

        ident_t = consts.tile([128, 128], BF16)
        nc.sync.dma_start(out=ident_t, in_=ident_d.ap())
        s16_t = consts.tile([16, 128], BF16)
        nc.sync.dma_start(out=s16_t, in_=s16_d.ap())
        sel16_t = consts.tile([128, 16], BF16)
        nc.sync.dma_start(out=sel16_t, in_=sel16_d.ap())
        rs_t = consts.tile([1, G4], BF16)
        nc.sync.dma_start(out=rs_t, in_=rs_d.ap())

        def bcast128(dram_handle):
            a = dram_handle.ap()
            return bass.AP(tensor=a.tensor, offset=a.offset,
                           ap=[[0, 128], list(a.ap[-1])])

        whh_t = wpool.tile([128, 4 * G4], BF16)
        nc.sync.dma_start(out=whh_t, in_=whh_d.ap())

        u_flat = u_d.ap().rearrange("b t d -> (b t) d")
        # ---------------- precompute P = u @ wih.T + bias (b dir only) -------
        with tc.tile_pool(name="pre_sb", bufs=3) as pre_sb, \
             tc.tile_pool(name="pre_w", bufs=1) as pre_w, \
             tc.tile_pool(name="pre_ps", bufs=2, space="PSUM") as pre_ps, \
             tc.tile_pool(name="pre_gps", bufs=1, space="PSUM") as pre_gps:
            wih_t = pre_w.tile([128, 4 * G4], BF16)
            nc.sync.dma_start(out=wih_t, in_=wih_d.ap())
            bias_t = pre_w.tile([128, G4], F32)
            nc.sync.dma_start(out=bias_t, in_=bcast128(bias_d))

            # (T, 16, 512): row 4q+b holds P'[t, b, quarter q]
            p_store = pb_d.ap().rearrange("t (q b) n -> t q b n", b=BL)

            def token_ranges(m):
                """Split token chunk m into per-batch-row runs.
                Yields (row0, b, t0, t1)."""
                lo = 128 * m
                hi = min(lo + 128, ntok)
                t0g = lo
                while t0g < hi:
                    b = t0g // T
                    t1g = min(hi, (b + 1) * T)
                    yield t0g - lo, b, t0g - b * T, t1g - b * T
                    t0g = t1g
            for m in range(nchunk):
                P = min(128, ntok - 128 * m)
                u_sb = pre_sb.tile([128, D], F32, tag="u_raw")
                nc.sync.dma_start(out=u_sb[:P], in_=u_flat[128 * m:128 * m + P, :])
                u_bf = pre_sb.tile([128, D], BF16, tag="u_bf")
                nc.vector.tensor_copy(u_bf[:P], u_sb[:P])
                ut_ps = pre_ps.tile([128, 512], BF16, tag="ut_ps")
                for k in range(4):
                    nc.tensor.transpose(ut_ps[:, 128 * k:128 * k + P],
                                        u_bf[:P, 128 * k:128 * (k + 1)],
                                        ident_t[:P, :P])
                ut_sb = pre_sb.tile([128, 512], BF16, tag="ut_sb")
                nc.vector.tensor_copy(ut_sb, ut_ps)
                g_ps = pre_gps.tile([128, G4], F32, tag="pre_g")
                for k in range(4):
                    for n in range(4):
                        nc.tensor.matmul(
                            g_ps[:P, 512 * n:512 * (n + 1)],
                            ut_sb[:, 128 * k:128 * k + P],
                            wih_t[:, G4 * k + 512 * n:G4 * k + 512 * (n + 1)],
                            start=(k == 0), stop=(k == 3))
                g_bf = pre_sb.tile([128, G4], BF16, tag="pre_o")
                nc.vector.scalar_tensor_tensor(
                    g_bf[:P], g_ps[:P], 1.0, bias_t[:P],
                    op0=OP.mult, op1=OP.add)
                for r0, b_, t0, t1 in token_ranges(m):
                    nc.sync.dma_start(
                        out=p_store[t0:t1, :, b_, :],
                        in_=g_bf[r0:r0 + (t1 - t0)].rearrange(
                            "r (q n) -> r q n", q=4))

        # ---------------- recurrence ----------------------------------------
        state = ctx.enter_context(tc.tile_pool(name="state", bufs=1))
        hT = state.tile([128, 16], BF16)
        cb = state.tile([128, 128], F32)
        xh_al = state.tile([1, 4], BF16)
        xh_s = state.tile([1, ntok], F32)
        xhn_s = state.tile([1, ntok], F32)
        for t_ in (hT, cb, xh_al):
            nc.vector.memset(t_, 0.0)

        warm = consts.tile([1, 4], BF16)
        nc.scalar.activation(warm, hT[0:1, 0:4], AF.Sigmoid)
        nc.scalar.activation(warm, hT[0:1, 0:4], AF.Tanh)
        ppool = ctx.enter_context(tc.tile_pool(name="prefetch", bufs=2))
        gps = ctx.enter_context(tc.tile_pool(name="gates_ps", bufs=3, space="PSUM"))
        hps = ctx.enter_context(tc.tile_pool(name="ht_ps", bufs=2, space="PSUM"))
        work = ctx.enter_context(tc.tile_pool(name="work", bufs=3))

        def cell(p_t, use_xh, xh_dst_expr):
            G = gps.tile([128, 512], F32, tag="G")
            nc.tensor.matmul(G, s16_t, p_t, start=True, stop=False,
                             skip_group_check=True)
            for k in range(4):
                for j in range(4):
                    nc.tensor.matmul(
                        G[32 * j:32 * j + 4, :],
                        hT[:, 4 * k:4 * k + 4],
                        whh_t[:, G4 * k + 512 * j:G4 * k + 512 * (j + 1)],
                        start=False,
                        stop=(not use_xh) and k == 3,
                        tile_position=(0, 32 * j),
                        skip_group_check=True)
            if use_xh:
                for j in range(4):
                    nc.tensor.matmul(G[32 * j:32 * j + 4, :], xh_al[0:1, 0:4],
                                     rs_t[0:1, 512 * j:512 * (j + 1)],
                                     start=False, stop=True,
                                     tile_position=(0, 32 * j),
                                     skip_group_check=True)
            sig = work.tile([128, 512], BF16, tag="sig")
            nc.scalar.activation(sig[:, 0:384], G[:, 0:384], AF.Sigmoid)
            nc.scalar.activation(sig[:, 384:512], G[:, 384:512], AF.Sigmoid)
            qt = work.tile([128, 128], BF16, tag="qt")
            nc.vector.tensor_scalar(qt, sig[:, 256:384], 2.0, 1.0,
                                    OP.mult, OP.subtract)
            t2 = work.tile([128, 128], BF16, tag="t2")
            nc.vector.tensor_tensor(t2, sig[:, 0:128], qt, OP.mult)
            t1 = work.tile([128, 128], F32, tag="t1")
            nc.vector.tensor_tensor(t1, sig[:, 128:256], cb, OP.mult)
            nc.vector.tensor_tensor(cb, t1, t2, OP.add)
            tc2 = work.tile([128, 128], BF16, tag="tc")
            nc.scalar.activation(tc2, cb, AF.Tanh)
            h2 = work.tile([128, 128], BF16, tag="h2")
            nc.vector.tensor_tensor(h2, sig[:, 384:512], tc2, OP.mult)
            hT_ps = hps.tile([128, 16], F32, tag="hTp")
            nc.tensor.matmul(hT_ps, h2, sel16_t, start=True, stop=True,
                             skip_group_check=True)
            nc.vector.tensor_copy(hT, hT_ps)
            xh_ps = hps.tile([1, 4], F32, tag="xhp")
            nc.tensor.matmul(xh_ps, h2[96:100, 127:128],
                             sel16_t[96:100, 12:16], start=True, stop=True,
                             tile_position=(96, 0), skip_group_check=True)
            nc.vector.tensor_copy(xh_al, xh_ps)
            nc.vector.tensor_copy(xh_dst_expr, xh_ps)

        def step(t_expr, pb_t=None):
            if pb_t is None:
                pb_t = ppool.tile([16, 512], BF16, tag="pb")
                nc.sync.dma_start(out=pb_t,
                                  in_=pb_d.ap()[ds(t_expr, 1)].squeeze(0))
            cell(pb_t, False, xh_s[0:1, ds(t_expr * 4, 4)])
            cell(pb_t, True, xhn_s[0:1, ds(t_expr * 4, 4)])

        if use_loop:
            assert T % unroll == 0
            with tc.For_i(0, T // unroll, 1,
                          hint_engines=tuple(mybir.ALL_ENGINES)) as i:
                slab_b = ppool.tile([16, 512 * unroll], BF16, tag="slabb")
                nc.sync.dma_start(
                    out=slab_b,
                    in_=pb_d.ap()[ds(i * unroll, unroll)].transpose([1, 0, 2]))
                for s_ in range(unroll):
                    step(i * unroll + s_,
                         slab_b[:, 512 * s_:512 * (s_ + 1)])
        else:
            for t_ in range(T):
                step(t_)

        # ---------------- output pass ---------------------------------------
        nc.vector.tensor_tensor(xh_s, xh_s, xhn_s, OP.add)
        nc.vector.tensor_scalar(xh_s, xh_s, 0.5, None, OP.mult)
        nc.sync.dma_start(out=s_d.ap().unsqueeze(0), in_=xh_s[0:1, :])
        s_bm = s_d.ap().rearrange("(t b) -> t b", b=BL).transpose([1, 0])
        out_flat = out_d.ap().rearrange("b t d -> (b t) d")

        def token_ranges2(m):
            lo, hi = 128 * m, min(128 * m + 128, ntok)
            t0g = lo
            while t0g < hi:
                b = t0g // T
                t1g = min(hi, (b + 1) * T)
                yield t0g - lo, b, t0g - b * T, t1g - b * T
                t0g = t1g

        with tc.tile_pool(name="post", bufs=4) as post:
            for m in range(nchunk):
                P = min(128, ntok - 128 * m)
                u_sb = post.tile([128, D], F32, tag="u_post")
                nc.sync.dma_start(out=u_sb[:P], in_=u_flat[128 * m:128 * m + P, :])
                s_pp = post.tile([128, 1], F32, tag="s_pp")
                for r0, b_, t0, t1 in token_ranges2(m):
                    nc.sync.dma_start(
                        out=s_pp[r0:r0 + (t1 - t0)],
                        in_=s_bm[b_, t0:t1].unsqueeze(1))
                o_sb = post.tile([128, D], F32, tag="o_post")
                nc.vector.tensor_scalar(o_sb[:P], u_sb[:P], 1.5, s_pp[:P],
                                        OP.mult, OP.add)
                nc.sync.dma_start(out=out_flat[128 * m:128 * m + P, :],
                                  in_=o_sb[:P])

    nc.finalize()
    return nc


# ------------------------------------------------------------------- runner --

_CACHE = {}


def _get_program(T, unroll, use_loop):
    key = (T, unroll, use_loop)
    if key not in _CACHE:
        _CACHE[key] = build_program(T, unroll=unroll, use_loop=use_loop)
    return _CACHE[key]


_PJRT_CACHE = {}


def _run_pjrt(nc, in_maps, time_iters=0):
    """Execute via PJRT shard_map, keeping the jitted callable so repeated
    timed executions reuse staged inputs. Returns (results_list, best_ns)."""
    import time as _time
    import jax
    from jax.sharding import Mesh, PartitionSpec
    from jax.experimental.shard_map import shard_map
    import concourse.mybir as _mb
    from concourse import bass2jax as b2j

    b2j.install_neuronx_cc_hook()
    n_cores = len(in_maps)
    partition_name = nc.partition_id_tensor.name if nc.partition_id_tensor else None
    in_names, out_names, out_avals, zero_outs = [], [], [], []
    for alloc in nc.m.functions[0].allocations:
        if not isinstance(alloc, _mb.MemoryLocationSet):
            continue
        name = alloc.memorylocations[0].name
        if alloc.kind == "ExternalInput":
            if name != partition_name:
                in_names.append(name)
        elif alloc.kind == "ExternalOutput":
            shape = tuple(alloc.tensor_shape)
            dtype = _mb.dt.np(alloc.dtype)
            out_names.append(name)
            out_avals.append(jax.core.ShapedArray(shape, dtype))
            zero_outs.append(np.zeros(shape, dtype))
    n_params = len(in_names)
    all_in = list(in_names) + list(out_names)
    if partition_name is not None:
        all_in.append(partition_name)

    def _body(*args):
        operands = list(args)
        if partition_name is not None:
            operands.append(b2j.partition_id_tensor())
        outs = b2j._bass_exec_p.bind(
            *operands, out_avals=tuple(out_avals), in_names=tuple(all_in),
            out_names=tuple(out_names), lowering_input_output_aliases=(),
            sim_require_finite=True, sim_require_nnan=True, nc=nc)
        return tuple(outs)

    cached = _PJRT_CACHE.get(id(nc))
    if cached is None:
        from jax.sharding import NamedSharding
        import jax.numpy as jnp
        devices = jax.devices()[:n_cores]
        mesh = Mesh(np.array(devices), ("core",))
        n_outs = len(out_names)
        sharded = jax.jit(
            shard_map(_body, mesh=mesh,
                      in_specs=(PartitionSpec("core"),) * (n_params + n_outs),
                      out_specs=(PartitionSpec("core"),) * n_outs,
                      check_rep=False),
            keep_unused=True)
        # Materialize zero output buffers directly on-device (sharded), so
        # they are never shipped from host on dispatch.
        zshapes = [((n_cores * z.shape[0], *z.shape[1:]), z.dtype)
                   for z in zero_outs]
        mk = jax.jit(
            lambda: tuple(jnp.zeros(s, d) for s, d in zshapes),
            out_shardings=tuple(
                NamedSharding(mesh, PartitionSpec("core"))
                for _ in zshapes))
        zero_args = list(jax.block_until_ready(mk()))
        _PJRT_CACHE[id(nc)] = (sharded, zero_args)
    else:
        sharded, zero_args = cached
    concat_in = [np.concatenate([np.asarray(in_maps[c][nm])
                                 for c in range(n_cores)], axis=0)
                 for nm in in_names]
    args = [jax.device_put(a) for a in concat_in] + zero_args
    out_arrs = jax.block_until_ready(sharded(*args))
    best = None
    all_ns = []
    for _ in range(time_iters):
        t0 = _time.perf_counter()
        out_arrs2 = jax.block_until_ready(sharded(*args))
        dt = _time.perf_counter() - t0
        all_ns.append(int(dt * 1e9))
        best = dt if best is None or dt < best else best
    kernel._all_ns = all_ns
    results = [{nm: np.asarray(out_arrs[i]).reshape(n_cores,
                                                    *out_avals[i].shape)[c]
                for i, nm in enumerate(out_names)}
               for c in range(n_cores)]
    return results, (None if best is None else int(best * 1e9))


def kernel(u_sequence, w_ih_f, w_hh_f, b_ih_f, b_hh_f,
           w_ih_b, w_hh_b, b_ih_b, b_hh_b, _time_iters=0):
    u = np.asarray(u_sequence, np.float32)
    Bn, T, Dn = u.shape
    assert (Bn, Dn) == (B, D)

    whh_i, wih_i, bias, rs = _prep_dir(w_ih_b, w_hh_b, b_ih_b, b_hh_b)
    ident = np.eye(128, dtype=_BF)
    s16 = np.zeros((16, 128), dtype=_BF)
    for q in range(4):
        for b_ in range(BL):
            s16[4 * q + b_, 32 * q + b_] = 1
    sel16 = np.ascontiguousarray(s16.T)

    unroll = 8
    use_loop = (T % unroll == 0) and T >= 16 \
        and not os.environ.get('KERNEL_NO_LOOP')
    nc = _get_program(T, unroll, use_loop)

    common = dict(whh_b=whh_i, wih_b=wih_i, bias_b=bias, rs_b=rs,
                  ident=ident, s16=s16, sel16=sel16)
    in_maps = []
    for c in range(N_CORES):
        m = dict(common)
        m["u"] = np.ascontiguousarray(u[c * BL:(c + 1) * BL])
        in_maps.append(m)

    results, best_ns = _run_pjrt(nc, in_maps, time_iters=_time_iters)
    out = np.concatenate([results[c]["out"] for c in range(N_CORES)], axis=0)
    kernel._last_ns = best_ns
    return out


if __name__ == "__main__":
    # tiny smoke: T=16 vs jax reference
    import reference
    T_s = 16
    inputs = reference.setup_inputs()
    inputs = {k: np.asarray(v) for k, v in inputs.items()}
    inputs["u_sequence"] = inputs["u_sequence"][:, :T_s, :]
    want = np.asarray(reference.reference(**inputs))
    got = kernel(**inputs)
    err = np.abs(got - want)
    rel = err.max() / np.abs(want).max()
    print(f"T={T_s}  maxabs={err.max():.3e}  rel={rel:.3e}")


# revision 35
# speedup vs baseline: 1.0189x; 1.0189x over previous
"""Trainium2 Bass kernel for nn_ExHuneLSTM (bidirectional single-step LSTM scan).

Key observation: the forward-direction LSTM in the reference is dead code.
``x_hat = out1[:, -1:]`` is the last channel of ``concat(hf, hb)`` — i.e.
``hb[:, -1]`` — and the backward cell never reads ``hf/cf``.  Only the
backward-parameter cell chain affects the output:

  per step t:  hb1, cb1 = cell_b(u_t, hb, cb)        xh  = hb1[:, 511]
               hb2, cb2 = cell_b(u_t + xh, hb1, cb1) xhn = hb2[:, 511]
               out[t] = 1.5*u_t + 0.5*(xh + xhn)

Implementation notes (per core, batch sharded 8 ways, B_local=4):
  * P[t] = u_t @ w_ih.T + (b_ih + b_hh) precomputed for all t with one big
    GEMM pass, streamed back per step. Phase B's input projection is
    P[t] + xh * rowsum(w_ih): a rank-1 correction folded into the gate
    accumulation (applied last so the matmuls can start early).
  * Gate pre-activations in PSUM as (128, 512): partition 32*q + b
    (q = H-quarter, b = batch), free = [i|f|g|o] x 128.  Weights are
    pre-permuted on host to quarter-major gate-interleaved order; g-gate
    columns pre-scaled by 2 so tanh(g) = 2*sigmoid(2g)-1 comes out of the
    same fused sigmoid.
  * Recurrent matmul: stationary = hT column slices (128, 4) from a compact
    transposed state hT (128, 16) (col 4k+b = quarter k, batch b); moving =
    w_hh.T chunks (128, 512) bf16 through 4 PE column groups concurrently
    (tile_position=(0, 32*j)).
  * h transpose uses a (128, 16) selector as the transpose identity, so the
    PE streams 16 columns instead of 128 and the PSUM->SBUF copy is (128,16).
    xh is then just hT[127, 12:16] — no separate extraction transpose.
"""

import math
import os
from contextlib import ExitStack

import numpy as np
import ml_dtypes

import concourse.bass as bass
import concourse.mybir as mybir
import concourse.tile as tile
from concourse import bacc
from concourse.bass import ds
from concourse.bass_utils import run_bass_kernel_spmd

F32 = mybir.dt.float32
BF16 = mybir.dt.bfloat16
AF = mybir.ActivationFunctionType
OP = mybir.AluOpType

B, D, H = 32, 512, 512
G4 = 4 * H  # 2048 gate dim
N_CORES = 8
BL = B // N_CORES  # 4 batch rows per core

_BF = ml_dtypes.bfloat16


# ---------------------------------------------------------------- host prep --

def _perm_rows(x):
    """Permute gate-dim (4H) from gate-major (g,q,h) to quarter-major
    (q,g,h) along axis 0."""
    s = x.shape
    y = x.reshape(4, 4, 128, *s[1:])
    return y.transpose(1, 0, 2, *range(3, y.ndim)).reshape(*s)


def _prep_dir(w_ih, w_hh, b_ih, b_hh):
    """Returns (whh_img (128, 8192) bf16, wih_img (128, 8192) bf16,
    bias (1, 2048) f32, rowsum (1, 2048) bf16) with gate permutation and
    2x pre-scale on the g gate."""
    w_ih = np.asarray(w_ih, np.float32).copy()
    w_hh = np.asarray(w_hh, np.float32).copy()
    bias = (np.asarray(b_ih, np.float32) + np.asarray(b_hh, np.float32)).copy()
    rowsum = w_ih.sum(axis=1)
    # 2x scale on g gate (PyTorch order i,f,g,o -> rows 1024:1536)
    w_ih[1024:1536] *= 2.0
    w_hh[1024:1536] *= 2.0
    bias = bias.copy(); bias[1024:1536] *= 2.0
    rowsum = rowsum.copy(); rowsum[1024:1536] *= 2.0
    w_ih = _perm_rows(w_ih)
    w_hh = _perm_rows(w_hh)
    bias = _perm_rows(bias)
    rowsum = _perm_rows(rowsum)

    def img(w):  # (2048, 512) -> w.T (512, 2048) -> K-chunks side by side
        wt = w.T.astype(_BF)                       # (512, 2048)
        return np.ascontiguousarray(
            wt.reshape(4, 128, G4).transpose(1, 0, 2).reshape(128, 4 * G4))

    return (img(w_hh), img(w_ih),
            bias.reshape(1, G4).astype(np.float32),
            rowsum.reshape(1, G4).astype(_BF))


# ------------------------------------------------------------ device program --

def build_program(T, unroll=8, use_loop=True):
    nc = bacc.Bacc("TRN2", num_devices=N_CORES, debug=False)

    u_d = nc.dram_tensor("u", (BL, T, D), F32, kind="ExternalInput")
    whh_d = nc.dram_tensor("whh_b", (128, 4 * G4), BF16, kind="ExternalInput")
    wih_d = nc.dram_tensor("wih_b", (128, 4 * G4), BF16, kind="ExternalInput")
    bias_d = nc.dram_tensor("bias_b", (1, G4), F32, kind="ExternalInput")
    rs_d = nc.dram_tensor("rs_b", (1, G4), BF16, kind="ExternalInput")
    ident_d = nc.dram_tensor("ident", (128, 128), BF16, kind="ExternalInput")
    s16_d = nc.dram_tensor("s16", (16, 128), BF16, kind="ExternalInput")
    sel16_d = nc.dram_tensor("sel16", (128, 16), BF16, kind="ExternalInput")
    aux_d = nc.dram_tensor("aux", (4, 256), BF16, kind="ExternalInput")
    out_d = nc.dram_tensor("out", (BL, T, D), F32, kind="ExternalOutput")

    pb_d = nc.dram_tensor("pb_scratch", (T, 16, 512), BF16, kind="Internal")
    s_d = nc.dram_tensor("s_scratch", (BL * T,), F32, kind="Internal")

    ntok = BL * T
    nchunk = math.ceil(ntok / 128)

    with ExitStack() as ctx:
        tc = ctx.enter_context(tile.TileContext(nc))

        consts = ctx.enter_context(tc.tile_pool(name="consts", bufs=1))
        wpool = ctx.enter_context(tc.tile_pool(name="weights", bufs=1))

        ident_t = consts.tile([128, 128], BF16)
        nc.sync.dma_start(out=ident_t, in_=ident_d.ap())
        s16_t = consts.tile([16, 128], BF16)
        nc.sync.dma_start(out=s16_t, in_=s16_d.ap())
        sel16_t = consts.tile([128, 16], BF16)
        nc.sync.dma_start(out=sel16_t, in_=sel16_d.ap())
        aux_t = consts.tile([4, 256], BF16)
        nc.sync.dma_start(out=aux_t, in_=aux_d.ap())
        rs4_t = consts.tile([4, 512], BF16)
        nc.sync.dma_start(out=rs4_t,
                          in_=rs_d.ap().rearrange("o (q n) -> (o q) n", q=4))

        def bcast128(dram_handle):
            a = dram_handle.ap()
            return bass.AP(tensor=a.tensor, offset=a.offset,
                           ap=[[0, 128], list(a.ap[-1])])

        whh_t = wpool.tile([128, 4 * G4], BF16)
        nc.sync.dma_start(out=whh_t, in_=whh_d.ap())

        u_flat = u_d.ap().rearrange("b t d -> (b t) d")
        # ---------------- precompute P = u @ wih.T + bias (b dir only) -------
        with tc.tile_pool(name="pre_sb", bufs=3) as pre_sb, \
             tc.tile_pool(name="pre_w", bufs=1) as pre_w, \
             tc.tile_pool(name="pre_ps", bufs=2, space="PSUM") as pre_ps, \
             tc.tile_pool(name="pre_gps", bufs=1, space="PSUM") as pre_gps:
            wih_t = pre_w.tile([128, 4 * G4], BF16)
            nc.sync.dma_start(out=wih_t, in_=wih_d.ap())
            bias_t = pre_w.tile([128, G4], F32)
            nc.sync.dma_start(out=bias_t, in_=bcast128(bias_d))

            # (T, 16, 512): row 4q+b holds P'[t, b, quarter q]
            p_store = pb_d.ap().rearrange("t (q b) n -> t q b n", b=BL)

            def token_ranges(m):
                """Split token chunk m into per-batch-row runs.
                Yields (row0, b, t0, t1)."""
                lo = 128 * m
                hi = min(lo + 128, ntok)
                t0g = lo
                while t0g < hi:
                    b = t0g // T
                    t1g = min(hi, (b + 1) * T)
                    yield t0g - lo, b, t0g - b * T, t1g - b * T
                    t0g = t1g
            for m in range(nchunk):
                P = min(128, ntok - 128 * m)
                u_sb = pre_sb.tile([128, D], F32, tag="u_raw")
                nc.sync.dma_start(out=u_sb[:P], in_=u_flat[128 * m:128 * m + P, :])
                u_bf = pre_sb.tile([128, D], BF16, tag="u_bf")
                nc.vector.tensor_copy(u_bf[:P], u_sb[:P])
                ut_ps = pre_ps.tile([128, 512], BF16, tag="ut_ps")
                for k in range(4):
                    nc.tensor.transpose(ut_ps[:, 128 * k:128 * k + P],
                                        u_bf[:P, 128 * k:128 * (k + 1)],
                                        ident_t[:P, :P])
                ut_sb = pre_sb.tile([128, 512], BF16, tag="ut_sb")
                nc.vector.tensor_copy(ut_sb, ut_ps)
                g_ps = pre_gps.tile([128, G4], F32, tag="pre_g")
                for k in range(4):
                    for n in range(4):
                        nc.tensor.matmul(
                            g_ps[:P, 512 * n:512 * (n + 1)],
                            ut_sb[:, 128 * k:128 * k + P],
                            wih_t[:, G4 * k + 512 * n:G4 * k + 512 * (n + 1)],
                            start=(k == 0), stop=(k == 3))
                g_bf = pre_sb.tile([128, G4], BF16, tag="pre_o")
                nc.vector.scalar_tensor_tensor(
                    g_bf[:P], g_ps[:P], 1.0, bias_t[:P],
                    op0=OP.mult, op1=OP.add)
                for r0, b_, t0, t1 in token_ranges(m):
                    nc.sync.dma_start(
                        out=p_store[t0:t1, :, b_, :],
                        in_=g_bf[r0:r0 + (t1 - t0)].rearrange(
                            "r (q n) -> r q n", q=4))

        # ---------------- recurrence ----------------------------------------
        state = ctx.enter_context(tc.tile_pool(name="state", bufs=1))
        hT = state.tile([128, 16], BF16)
        cb = state.tile([128, 128], F32)
        xh_al = state.tile([1, 4], BF16)
        xh_row = state.tile([1, 128], BF16)  # xh[b] at col 32q+b, else 0
        xh_s = state.tile([1, ntok], F32)
        xhn_s = state.tile([1, ntok], F32)
        for t_ in (hT, cb, xh_al, xh_row):
            nc.vector.memset(t_, 0.0)

        warm = consts.tile([1, 4], BF16)
        nc.scalar.activation(warm, hT[0:1, 0:4], AF.Sigmoid)
        nc.scalar.activation(warm, hT[0:1, 0:4], AF.Tanh)
        ppool = ctx.enter_context(tc.tile_pool(name="prefetch", bufs=2))
        gps = ctx.enter_context(tc.tile_pool(name="gates_ps", bufs=4, space="PSUM"))
        hps = ctx.enter_context(tc.tile_pool(name="ht_ps", bufs=1, space="PSUM"))
        work = ctx.enter_context(tc.tile_pool(name="work", bufs=3))

        def cell(p_t, use_xh, xh_dst_expr):
            G = gps.tile([128, 512], F32, tag="G")
            if use_xh:
                # xh4blk[c, 32q+b] = xh[b] * (c == q): scatter xh into a
                # strided row, broadcast across 4 partitions with a ones
                # column, mask to block-diagonal; feeds a single K=4
                # rank-1-per-quarter matmul that closes the accumulation.
                a_out = xh_row[0:1, :]
                out_ap = bass.AP(tensor=a_out.tensor, offset=a_out.offset,
                                 ap=[list(a_out.ap[0]), [32, 4], [1, 4]])
                a_in = xh_al[0:1, 0:4]
                in_ap = bass.AP(tensor=a_in.tensor, offset=a_in.offset,
                                ap=[list(a_in.ap[0]), [0, 4], [1, 4]])
                nc.vector.tensor_copy(out_ap, in_ap)
                xh4bc_ps = hps.tile([4, 128], F32, tag="xh4bc")
                nc.tensor.matmul(xh4bc_ps, aux_t[0:1, 128:132], xh_row,
                                 start=True, stop=True, skip_group_check=True)
                xh4blk = work.tile([4, 128], BF16, tag="xh4blk")
                nc.vector.tensor_tensor(xh4blk, xh4bc_ps, aux_t[0:4, 0:128],
                                        OP.mult)
            nc.tensor.matmul(G, s16_t, p_t, start=True, stop=False,
                             skip_group_check=True)
            for k in range(4):
                for j in range(4):
                    nc.tensor.matmul(
                        G[32 * j:32 * j + 4, :],
                        hT[:, 4 * k:4 * k + 4],
                        whh_t[:, G4 * k + 512 * j:G4 * k + 512 * (j + 1)],
                        start=False,
                        stop=(not use_xh) and k == 3,
                        tile_position=(0, 32 * j),
                        skip_group_check=True)
            if use_xh:
                nc.tensor.matmul(G, xh4blk, rs4_t, start=False, stop=True,
                                 skip_group_check=True)
            sig = work.tile([128, 512], BF16, tag="sig")
            nc.scalar.activation(sig[:, 0:384], G[:, 0:384], AF.Sigmoid)
            nc.scalar.activation(sig[:, 384:512], G[:, 384:512], AF.Sigmoid)
            qt = work.tile([128, 128], BF16, tag="qt")
            nc.vector.tensor_scalar(qt, sig[:, 256:384], 2.0, 1.0,
                                    OP.mult, OP.subtract)
            t2 = work.tile([128, 128], BF16, tag="t2")
            nc.vector.tensor_tensor(t2, sig[:, 0:128], qt, OP.mult)
            t1 = work.tile([128, 128], F32, tag="t1")
            nc.vector.tensor_tensor(t1, sig[:, 128:256], cb, OP.mult)
            nc.vector.tensor_tensor(cb, t1, t2, OP.add)
            tc2 = work.tile([128, 128], BF16, tag="tc")
            nc.scalar.activation(tc2, cb, AF.Tanh)
            h2 = work.tile([128, 128], BF16, tag="h2")
            nc.vector.tensor_tensor(h2, sig[:, 384:512], tc2, OP.mult)
            hT_ps = hps.tile([128, 16], F32, tag="hTp")
            nc.tensor.matmul(hT_ps, h2, sel16_t, start=True, stop=True,
                             skip_group_check=True)
            nc.vector.tensor_copy(hT, hT_ps)
            xh_ps = hps.tile([1, 4], F32, tag="xhp")
            nc.tensor.matmul(xh_ps, h2[96:100, 127:128],
                             sel16_t[96:100, 12:16], start=True, stop=True,
                             tile_position=(96, 0), skip_group_check=True)
            nc.vector.tensor_copy(xh_al, xh_ps)
            nc.vector.tensor_copy(xh_dst_expr, xh_ps)

        def step(t_expr, pb_t=None):
            if pb_t is None:
                pb_t = ppool.tile([16, 512], BF16, tag="pb")
                nc.sync.dma_start(out=pb_t,
                                  in_=pb_d.ap()[ds(t_expr, 1)].squeeze(0))
            cell(pb_t, False, xh_s[0:1, ds(t_expr * 4, 4)])
            cell(pb_t, True, xhn_s[0:1, ds(t_expr * 4, 4)])

        if use_loop:
            assert T % unroll == 0
            with tc.For_i(0, T // unroll, 1,
                          hint_engines=tuple(mybir.ALL_ENGINES)) as i:
                slab_b = ppool.tile([16, 512 * unroll], BF16, tag="slabb")
                nc.sync.dma_start(
                    out=slab_b,
                    in_=pb_d.ap()[ds(i * unroll, unroll)].transpose([1, 0, 2]))
                for s_ in range(unroll):
                    step(i * unroll + s_,
                         slab_b[:, 512 * s_:512 * (s_ + 1)])
        else:
            for t_ in range(T):
                step(t_)

        # ---------------- output pass ---------------------------------------
        nc.vector.tensor_tensor(xh_s, xh_s, xhn_s, OP.add)
        nc.vector.tensor_scalar(xh_s, xh_s, 0.5, None, OP.mult)
        nc.sync.dma_start(out=s_d.ap().unsqueeze(0), in_=xh_s[0:1, :])
        s_bm = s_d.ap().rearrange("(t b) -> t b", b=BL).transpose([1, 0])
        out_flat = out_d.ap().rearrange("b t d -> (b t) d")

        def token_ranges2(m):
            lo, hi = 128 * m, min(128 * m + 128, ntok)
            t0g = lo
            while t0g < hi:
                b = t0g // T
                t1g = min(hi, (b + 1) * T)
                yield t0g - lo, b, t0g - b * T, t1g - b * T
                t0g = t1g

        with tc.tile_pool(name="post", bufs=4) as post:
            for m in range(nchunk):
                P = min(128, ntok - 128 * m)
                u_sb = post.tile([128, D], F32, tag="u_post")
                nc.sync.dma_start(out=u_sb[:P], in_=u_flat[128 * m:128 * m + P, :])
                s_pp = post.tile([128, 1], F32, tag="s_pp")
                for r0, b_, t0, t1 in token_ranges2(m):
                    nc.sync.dma_start(
                        out=s_pp[r0:r0 + (t1 - t0)],
                        in_=s_bm[b_, t0:t1].unsqueeze(1))
                o_sb = post.tile([128, D], F32, tag="o_post")
                nc.vector.tensor_scalar(o_sb[:P], u_sb[:P], 1.5, s_pp[:P],
                                        OP.mult, OP.add)
                nc.sync.dma_start(out=out_flat[128 * m:128 * m + P, :],
                                  in_=o_sb[:P])

    nc.finalize()
    return nc


# ------------------------------------------------------------------- runner --

_CACHE = {}


def _get_program(T, unroll, use_loop):
    key = (T, unroll, use_loop)
    if key not in _CACHE:
        _CACHE[key] = build_program(T, unroll=unroll, use_loop=use_loop)
    return _CACHE[key]


_PJRT_CACHE = {}


def _run_pjrt(nc, in_maps, time_iters=0):
    """Execute via PJRT shard_map, keeping the jitted callable so repeated
    timed executions reuse staged inputs. Returns (results_list, best_ns)."""
    import time as _time
    import jax
    from jax.sharding import Mesh, PartitionSpec
    from jax.experimental.shard_map import shard_map
    import concourse.mybir as _mb
    from concourse import bass2jax as b2j

    b2j.install_neuronx_cc_hook()
    n_cores = len(in_maps)
    partition_name = nc.partition_id_tensor.name if nc.partition_id_tensor else None
    in_names, out_names, out_avals, zero_outs = [], [], [], []
    for alloc in nc.m.functions[0].allocations:
        if not isinstance(alloc, _mb.MemoryLocationSet):
            continue
        name = alloc.memorylocations[0].name
        if alloc.kind == "ExternalInput":
            if name != partition_name:
                in_names.append(name)
        elif alloc.kind == "ExternalOutput":
            shape = tuple(alloc.tensor_shape)
            dtype = _mb.dt.np(alloc.dtype)
            out_names.append(name)
            out_avals.append(jax.core.ShapedArray(shape, dtype))
            zero_outs.append(np.zeros(shape, dtype))
    n_params = len(in_names)
    all_in = list(in_names) + list(out_names)
    if partition_name is not None:
        all_in.append(partition_name)

    def _body(*args):
        operands = list(args)
        if partition_name is not None:
            operands.append(b2j.partition_id_tensor())
        outs = b2j._bass_exec_p.bind(
            *operands, out_avals=tuple(out_avals), in_names=tuple(all_in),
            out_names=tuple(out_names), lowering_input_output_aliases=(),
            sim_require_finite=True, sim_require_nnan=True, nc=nc)
        return tuple(outs)

    cached = _PJRT_CACHE.get(id(nc))
    if cached is None:
        from jax.sharding import NamedSharding
        import jax.numpy as jnp
        devices = jax.devices()[:n_cores]
        mesh = Mesh(np.array(devices), ("core",))
        n_outs = len(out_names)
        sharded = jax.jit(
            shard_map(_body, mesh=mesh,
                      in_specs=(PartitionSpec("core"),) * (n_params + n_outs),
                      out_specs=(PartitionSpec("core"),) * n_outs,
                      check_rep=False),
            keep_unused=True)
        # Materialize zero output buffers directly on-device (sharded), so
        # they are never shipped from host on dispatch.
        zshapes = [((n_cores * z.shape[0], *z.shape[1:]), z.dtype)
                   for z in zero_outs]
        mk = jax.jit(
            lambda: tuple(jnp.zeros(s, d) for s, d in zshapes),
            out_shardings=tuple(
                NamedSharding(mesh, PartitionSpec("core"))
                for _ in zshapes))
        zero_args = list(jax.block_until_ready(mk()))
        _PJRT_CACHE[id(nc)] = (sharded, zero_args)
    else:
        sharded, zero_args = cached
    concat_in = [np.concatenate([np.asarray(in_maps[c][nm])
                                 for c in range(n_cores)], axis=0)
                 for nm in in_names]
    args = [jax.device_put(a) for a in concat_in] + zero_args
    out_arrs = jax.block_until_ready(sharded(*args))
    best = None
    all_ns = []
    for _ in range(time_iters):
        t0 = _time.perf_counter()
        out_arrs2 = jax.block_until_ready(sharded(*args))
        dt = _time.perf_counter() - t0
        all_ns.append(int(dt * 1e9))
        best = dt if best is None or dt < best else best
    kernel._all_ns = all_ns
    results = [{nm: np.asarray(out_arrs[i]).reshape(n_cores,
                                                    *out_avals[i].shape)[c]
                for i, nm in enumerate(out_names)}
               for c in range(n_cores)]
    return results, (None if best is None else int(best * 1e9))


def kernel(u_sequence, w_ih_f, w_hh_f, b_ih_f, b_hh_f,
           w_ih_b, w_hh_b, b_ih_b, b_hh_b, _time_iters=0):
    u = np.asarray(u_sequence, np.float32)
    Bn, T, Dn = u.shape
    assert (Bn, Dn) == (B, D)

    whh_i, wih_i, bias, rs = _prep_dir(w_ih_b, w_hh_b, b_ih_b, b_hh_b)
    ident = np.eye(128, dtype=_BF)
    s16 = np.zeros((16, 128), dtype=_BF)
    for q in range(4):
        for b_ in range(BL):
            s16[4 * q + b_, 32 * q + b_] = 1
    sel16 = np.ascontiguousarray(s16.T)
    aux = np.zeros((4, 256), dtype=_BF)
    aux[0, 128:256] = 1                       # ones row for xh broadcast
    for q in range(4):
        for b_ in range(BL):
            aux[q, 32 * q + b_] = 1           # block-diag selector mask

    unroll = 8
    use_loop = (T % unroll == 0) and T >= 16 \
        and not os.environ.get('KERNEL_NO_LOOP')
    nc = _get_program(T, unroll, use_loop)

    common = dict(whh_b=whh_i, wih_b=wih_i, bias_b=bias, rs_b=rs,
                  ident=ident, s16=s16, sel16=sel16, aux=aux)
    in_maps = []
    for c in range(N_CORES):
        m = dict(common)
        m["u"] = np.ascontiguousarray(u[c * BL:(c + 1) * BL])
        in_maps.append(m)

    results, best_ns = _run_pjrt(nc, in_maps, time_iters=_time_iters)
    out = np.concatenate([results[c]["out"] for c in range(N_CORES)], axis=0)
    kernel._last_ns = best_ns
    return out


if __name__ == "__main__":
    # tiny smoke: T=16 vs jax reference
    import reference
    T_s = 16
    inputs = reference.setup_inputs()
    inputs = {k: np.asarray(v) for k, v in inputs.items()}
    inputs["u_sequence"] = inputs["u_sequence"][:, :T_s, :]
    want = np.asarray(reference.reference(**inputs))
    got = kernel(**inputs)
    err = np.abs(got - want)
    rel = err.max() / np.abs(want).max()
    print(f"T={T_s}  maxabs={err.max():.3e}  rel={rel:.3e}")


# revision 36
# speedup vs baseline: 1.0913x; 1.0710x over previous
"""Trainium2 Bass kernel for nn_ExHuneLSTM (bidirectional single-step LSTM scan).

Key observation: the forward-direction LSTM in the reference is dead code.
``x_hat = out1[:, -1:]`` is the last channel of ``concat(hf, hb)`` — i.e.
``hb[:, -1]`` — and the backward cell never reads ``hf/cf``.  Only the
backward-parameter cell chain affects the output:

  per step t:  hb1, cb1 = cell_b(u_t, hb, cb)        xh  = hb1[:, 511]
               hb2, cb2 = cell_b(u_t + xh, hb1, cb1) xhn = hb2[:, 511]
               out[t] = 1.5*u_t + 0.5*(xh + xhn)

Implementation notes (per core, batch sharded 8 ways, B_local=4):
  * P[t] = u_t @ w_ih.T + (b_ih + b_hh) precomputed for all t with one big
    GEMM pass, streamed back per step. Phase B's input projection is
    P[t] + xh * rowsum(w_ih): a rank-1 correction folded into the gate
    accumulation (applied last so the matmuls can start early).
  * Gate pre-activations in PSUM as (128, 512): partition 32*q + b
    (q = H-quarter, b = batch), free = [i|f|g|o] x 128.  Weights are
    pre-permuted on host to quarter-major gate-interleaved order; g-gate
    columns pre-scaled by 2 so tanh(g) = 2*sigmoid(2g)-1 comes out of the
    same fused sigmoid.
  * Recurrent matmul: stationary = hT column slices (128, 4) from a compact
    transposed state hT (128, 16) (col 4k+b = quarter k, batch b); moving =
    w_hh.T chunks (128, 512) bf16 through 4 PE column groups concurrently
    (tile_position=(0, 32*j)).
  * h transpose uses a (128, 16) selector as the transpose identity, so the
    PE streams 16 columns instead of 128 and the PSUM->SBUF copy is (128,16).
    xh is then just hT[127, 12:16] — no separate extraction transpose.
"""

import math
import os
from contextlib import ExitStack

import numpy as np
import ml_dtypes

import concourse.bass as bass
import concourse.mybir as mybir
import concourse.tile as tile
from concourse import bacc
from concourse.bass import ds
from concourse.bass_utils import run_bass_kernel_spmd

F32 = mybir.dt.float32
BF16 = mybir.dt.bfloat16
AF = mybir.ActivationFunctionType
OP = mybir.AluOpType

B, D, H = 32, 512, 512
G4 = 4 * H  # 2048 gate dim
N_CORES = 8
BL = B // N_CORES  # 4 batch rows per core

_BF = ml_dtypes.bfloat16


# ---------------------------------------------------------------- host prep --

def _perm_rows(x):
    """Permute gate-dim (4H) from gate-major (g,q,h) to quarter-major
    (q,g,h) along axis 0."""
    s = x.shape
    y = x.reshape(4, 4, 128, *s[1:])
    return y.transpose(1, 0, 2, *range(3, y.ndim)).reshape(*s)


def _prep_dir(w_ih, w_hh, b_ih, b_hh):
    """Returns (whh_img (128, 8192) bf16, wih_img (128, 8192) bf16,
    bias (1, 2048) f32, rowsum (1, 2048) bf16) with gate permutation and
    2x pre-scale on the g gate."""
    w_ih = np.asarray(w_ih, np.float32).copy()
    w_hh = np.asarray(w_hh, np.float32).copy()
    bias = (np.asarray(b_ih, np.float32) + np.asarray(b_hh, np.float32)).copy()
    rowsum = w_ih.sum(axis=1)
    # 2x scale on g gate (PyTorch order i,f,g,o -> rows 1024:1536)
    w_ih[1024:1536] *= 2.0
    w_hh[1024:1536] *= 2.0
    bias = bias.copy(); bias[1024:1536] *= 2.0
    rowsum = rowsum.copy(); rowsum[1024:1536] *= 2.0
    w_ih = _perm_rows(w_ih)
    w_hh = _perm_rows(w_hh)
    bias = _perm_rows(bias)
    rowsum = _perm_rows(rowsum)

    def img(w):  # (2048, 512) -> w.T (512, 2048) -> K-chunks side by side
        wt = w.T.astype(_BF)                       # (512, 2048)
        return np.ascontiguousarray(
            wt.reshape(4, 128, G4).transpose(1, 0, 2).reshape(128, 4 * G4))

    return (img(w_hh), img(w_ih),
            bias.reshape(1, G4).astype(np.float32),
            rowsum.reshape(1, G4).astype(_BF))


# ------------------------------------------------------------ device program --

def build_program(T, unroll=8, use_loop=True):
    nc = bacc.Bacc("TRN2", num_devices=N_CORES, debug=False)

    u_d = nc.dram_tensor("u", (BL, T, D), F32, kind="ExternalInput")
    whh_d = nc.dram_tensor("whh_b", (128, 4 * G4), BF16, kind="ExternalInput")
    wih_d = nc.dram_tensor("wih_b", (128, 4 * G4), BF16, kind="ExternalInput")
    bias_d = nc.dram_tensor("bias_b", (1, G4), F32, kind="ExternalInput")
    rs_d = nc.dram_tensor("rs_b", (1, G4), BF16, kind="ExternalInput")
    ident_d = nc.dram_tensor("ident", (128, 128), BF16, kind="ExternalInput")
    s16_d = nc.dram_tensor("s16", (16, 128), BF16, kind="ExternalInput")
    sel16_d = nc.dram_tensor("sel16", (128, 16), BF16, kind="ExternalInput")
    aux_d = nc.dram_tensor("aux", (4, 256), BF16, kind="ExternalInput")
    out_d = nc.dram_tensor("out", (BL, T, D), F32, kind="ExternalOutput")

    pb_d = nc.dram_tensor("pb_scratch", (T, 16, 512), BF16, kind="Internal")
    s_d = nc.dram_tensor("s_scratch", (BL * T,), F32, kind="Internal")

    ntok = BL * T
    nchunk = math.ceil(ntok / 128)

    with ExitStack() as ctx:
        tc = ctx.enter_context(tile.TileContext(nc))

        consts = ctx.enter_context(tc.tile_pool(name="consts", bufs=1))
        wpool = ctx.enter_context(tc.tile_pool(name="weights", bufs=1))

        ident_t = consts.tile([128, 128], BF16)
        nc.sync.dma_start(out=ident_t, in_=ident_d.ap())
        s16_t = consts.tile([16, 128], BF16)
        nc.sync.dma_start(out=s16_t, in_=s16_d.ap())
        sel16_t = consts.tile([128, 16], BF16)
        nc.sync.dma_start(out=sel16_t, in_=sel16_d.ap())
        aux_t = consts.tile([4, 256], BF16)
        nc.sync.dma_start(out=aux_t, in_=aux_d.ap())
        rs4_t = consts.tile([4, 512], BF16)
        nc.sync.dma_start(out=rs4_t,
                          in_=rs_d.ap().rearrange("o (q n) -> (o q) n", q=4))

        def bcast128(dram_handle):
            a = dram_handle.ap()
            return bass.AP(tensor=a.tensor, offset=a.offset,
                           ap=[[0, 128], list(a.ap[-1])])

        whh_t = wpool.tile([128, 4 * G4], BF16)
        nc.sync.dma_start(out=whh_t, in_=whh_d.ap())

        u_flat = u_d.ap().rearrange("b t d -> (b t) d")
        # ---------------- precompute P = u @ wih.T + bias (b dir only) -------
        with tc.tile_pool(name="pre_sb", bufs=3) as pre_sb, \
             tc.tile_pool(name="pre_w", bufs=1) as pre_w, \
             tc.tile_pool(name="pre_ps", bufs=2, space="PSUM") as pre_ps, \
             tc.tile_pool(name="pre_gps", bufs=1, space="PSUM") as pre_gps:
            wih_t = pre_w.tile([128, 4 * G4], BF16)
            nc.sync.dma_start(out=wih_t, in_=wih_d.ap())
            bias_t = pre_w.tile([128, G4], F32)
            nc.sync.dma_start(out=bias_t, in_=bcast128(bias_d))

            # (T, 16, 512): row 4q+b holds P'[t, b, quarter q]
            p_store = pb_d.ap().rearrange("t (q b) n -> t q b n", b=BL)

            def token_ranges(m):
                """Split token chunk m into per-batch-row runs.
                Yields (row0, b, t0, t1)."""
                lo = 128 * m
                hi = min(lo + 128, ntok)
                t0g = lo
                while t0g < hi:
                    b = t0g // T
                    t1g = min(hi, (b + 1) * T)
                    yield t0g - lo, b, t0g - b * T, t1g - b * T
                    t0g = t1g
            for m in range(nchunk):
                P = min(128, ntok - 128 * m)
                u_sb = pre_sb.tile([128, D], F32, tag="u_raw")
                nc.sync.dma_start(out=u_sb[:P], in_=u_flat[128 * m:128 * m + P, :])
                u_bf = pre_sb.tile([128, D], BF16, tag="u_bf")
                nc.vector.tensor_copy(u_bf[:P], u_sb[:P])
                ut_ps = pre_ps.tile([128, 512], BF16, tag="ut_ps")
                for k in range(4):
                    nc.tensor.transpose(ut_ps[:, 128 * k:128 * k + P],
                                        u_bf[:P, 128 * k:128 * (k + 1)],
                                        ident_t[:P, :P])
                ut_sb = pre_sb.tile([128, 512], BF16, tag="ut_sb")
                nc.vector.tensor_copy(ut_sb, ut_ps)
                g_ps = pre_gps.tile([128, G4], F32, tag="pre_g")
                for k in range(4):
                    for n in range(4):
                        nc.tensor.matmul(
                            g_ps[:P, 512 * n:512 * (n + 1)],
                            ut_sb[:, 128 * k:128 * k + P],
                            wih_t[:, G4 * k + 512 * n:G4 * k + 512 * (n + 1)],
                            start=(k == 0), stop=(k == 3))
                g_bf = pre_sb.tile([128, G4], BF16, tag="pre_o")
                nc.vector.scalar_tensor_tensor(
                    g_bf[:P], g_ps[:P], 1.0, bias_t[:P],
                    op0=OP.mult, op1=OP.add)
                for r0, b_, t0, t1 in token_ranges(m):
                    nc.sync.dma_start(
                        out=p_store[t0:t1, :, b_, :],
                        in_=g_bf[r0:r0 + (t1 - t0)].rearrange(
                            "r (q n) -> r q n", q=4))

        # ---------------- recurrence ----------------------------------------
        state = ctx.enter_context(tc.tile_pool(name="state", bufs=1))
        hT = state.tile([128, 16], BF16)
        cb = state.tile([128, 128], F32)
        xh_al = state.tile([1, 4], BF16)
        xh_row = state.tile([1, 128], BF16)  # xh[b] at col 32q+b, else 0
        xh_s = state.tile([1, ntok], F32)
        xhn_s = state.tile([1, ntok], F32)
        for t_ in (hT, cb, xh_al, xh_row):
            nc.vector.memset(t_, 0.0)

        warm = consts.tile([1, 4], BF16)
        nc.scalar.activation(warm, hT[0:1, 0:4], AF.Sigmoid)
        nc.scalar.activation(warm, hT[0:1, 0:4], AF.Tanh)
        ppool = ctx.enter_context(tc.tile_pool(name="prefetch", bufs=2))
        gps = ctx.enter_context(tc.tile_pool(name="gates_ps", bufs=4, space="PSUM"))
        hps = ctx.enter_context(tc.tile_pool(name="ht_ps", bufs=1, space="PSUM"))
        work = ctx.enter_context(tc.tile_pool(name="work", bufs=3))

        def cell(p_t, use_xh, xh_dst_expr):
            G = gps.tile([128, 512], F32, tag="G")
            if use_xh:
                # xh4blk[c, 32q+b] = xh[b] * (c == q): scatter xh into a
                # strided row, broadcast across 4 partitions with a ones
                # column, mask to block-diagonal; feeds a single K=4
                # rank-1-per-quarter matmul that closes the accumulation.
                a_out = xh_row[0:1, :]
                out_ap = bass.AP(tensor=a_out.tensor, offset=a_out.offset,
                                 ap=[list(a_out.ap[0]), [32, 4], [1, 4]])
                a_in = xh_al[0:1, 0:4]
                in_ap = bass.AP(tensor=a_in.tensor, offset=a_in.offset,
                                ap=[list(a_in.ap[0]), [0, 4], [1, 4]])
                nc.vector.tensor_copy(out_ap, in_ap)
                xh4bc_ps = hps.tile([4, 128], F32, tag="xh4bc")
                nc.tensor.matmul(xh4bc_ps, aux_t[0:1, 128:132], xh_row,
                                 start=True, stop=True, skip_group_check=True)
                xh4blk = work.tile([4, 128], BF16, tag="xh4blk")
                nc.vector.tensor_tensor(xh4blk, xh4bc_ps, aux_t[0:4, 0:128],
                                        OP.mult)
            nc.tensor.matmul(G, s16_t, p_t, start=True, stop=False,
                             skip_group_check=True)
            for k in range(4):
                for j in range(4):
                    nc.tensor.matmul(
                        G[32 * j:32 * j + 4, :],
                        hT[:, 4 * k:4 * k + 4],
                        whh_t[:, G4 * k + 512 * j:G4 * k + 512 * (j + 1)],
                        start=False,
                        stop=(not use_xh) and k == 3,
                        tile_position=(0, 32 * j),
                        skip_group_check=True)
            if use_xh:
                nc.tensor.matmul(G, xh4blk, rs4_t, start=False, stop=True,
                                 skip_group_check=True)
            sig = work.tile([128, 512], BF16, tag="sig")
            nc.scalar.activation(sig[:, 0:384], G[:, 0:384], AF.Sigmoid)
            nc.scalar.activation(sig[:, 384:512], G[:, 384:512], AF.Sigmoid)
            qt = work.tile([128, 128], BF16, tag="qt")
            nc.vector.tensor_scalar(qt, sig[:, 256:384], 2.0, 1.0,
                                    OP.mult, OP.subtract)
            t2 = work.tile([128, 128], BF16, tag="t2")
            nc.vector.tensor_tensor(t2, sig[:, 0:128], qt, OP.mult)
            t1 = work.tile([128, 128], F32, tag="t1")
            nc.vector.tensor_tensor(t1, sig[:, 128:256], cb, OP.mult)
            nc.vector.tensor_tensor(cb, t1, t2, OP.add)
            tc2 = work.tile([128, 128], BF16, tag="tc")
            nc.scalar.activation(tc2, cb, AF.Tanh)
            h2 = work.tile([128, 128], BF16, tag="h2")
            nc.vector.tensor_tensor(h2, sig[:, 384:512], tc2, OP.mult)
            hT_ps = hps.tile([128, 16], F32, tag="hTp")
            nc.tensor.matmul(hT_ps, h2, sel16_t, start=True, stop=True,
                             skip_group_check=True)
            nc.vector.tensor_copy(hT, hT_ps)
            xh_ps = hps.tile([1, 4], F32, tag="xhp")
            nc.tensor.matmul(xh_ps, h2[96:100, 127:128],
                             sel16_t[96:100, 12:16], start=True, stop=True,
                             tile_position=(96, 0), skip_group_check=True)
            nc.vector.tensor_copy(xh_al, xh_ps)
            nc.vector.tensor_copy(xh_dst_expr, xh_ps)

        def step(t_expr, pb_t=None):
            if pb_t is None:
                pb_t = ppool.tile([16, 512], BF16, tag="pb")
                nc.sync.dma_start(out=pb_t,
                                  in_=pb_d.ap()[ds(t_expr, 1)].squeeze(0))
            cell(pb_t, False, xh_s[0:1, ds(t_expr * 4, 4)])
            cell(pb_t, True, xhn_s[0:1, ds(t_expr * 4, 4)])

        if use_loop:
            assert T % unroll == 0
            with tc.For_i(0, T // unroll, 1,
                          hint_engines=tuple(mybir.ALL_ENGINES)) as i:
                slab_b = ppool.tile([16, 512 * unroll], BF16, tag="slabb")
                nc.sync.dma_start(
                    out=slab_b,
                    in_=pb_d.ap()[ds(i * unroll, unroll)].transpose([1, 0, 2]))
                for s_ in range(unroll):
                    step(i * unroll + s_,
                         slab_b[:, 512 * s_:512 * (s_ + 1)])
        else:
            for t_ in range(T):
                step(t_)

        # ---------------- output pass ---------------------------------------
        nc.vector.tensor_tensor(xh_s, xh_s, xhn_s, OP.add)
        nc.vector.tensor_scalar(xh_s, xh_s, 0.5, None, OP.mult)
        nc.sync.dma_start(out=s_d.ap().unsqueeze(0), in_=xh_s[0:1, :])
        s_bm = s_d.ap().rearrange("(t b) -> t b", b=BL).transpose([1, 0])
        out_flat = out_d.ap().rearrange("b t d -> (b t) d")

        def token_ranges2(m):
            lo, hi = 128 * m, min(128 * m + 128, ntok)
            t0g = lo
            while t0g < hi:
                b = t0g // T
                t1g = min(hi, (b + 1) * T)
                yield t0g - lo, b, t0g - b * T, t1g - b * T
                t0g = t1g

        with tc.tile_pool(name="post", bufs=4) as post:
            for m in range(nchunk):
                P = min(128, ntok - 128 * m)
                u_sb = post.tile([128, D], F32, tag="u_post")
                nc.sync.dma_start(out=u_sb[:P], in_=u_flat[128 * m:128 * m + P, :])
                s_pp = post.tile([128, 1], F32, tag="s_pp")
                for r0, b_, t0, t1 in token_ranges2(m):
                    nc.sync.dma_start(
                        out=s_pp[r0:r0 + (t1 - t0)],
                        in_=s_bm[b_, t0:t1].unsqueeze(1))
                o_sb = post.tile([128, D], F32, tag="o_post")
                nc.vector.tensor_scalar(o_sb[:P], u_sb[:P], 1.5, s_pp[:P],
                                        OP.mult, OP.add)
                nc.sync.dma_start(out=out_flat[128 * m:128 * m + P, :],
                                  in_=o_sb[:P])

    nc.finalize()
    return nc


# ------------------------------------------------------------------- runner --

_CACHE = {}


def _get_program(T, unroll, use_loop):
    key = (T, unroll, use_loop)
    if key not in _CACHE:
        _CACHE[key] = build_program(T, unroll=unroll, use_loop=use_loop)
    return _CACHE[key]


_PJRT_CACHE = {}


def _run_pjrt(nc, in_maps, time_iters=0):
    """Execute via PJRT shard_map, keeping the jitted callable so repeated
    timed executions reuse staged inputs. Returns (results_list, best_ns)."""
    import time as _time
    import jax
    from jax.sharding import Mesh, PartitionSpec
    from jax.experimental.shard_map import shard_map
    import concourse.mybir as _mb
    from concourse import bass2jax as b2j

    b2j.install_neuronx_cc_hook()
    n_cores = len(in_maps)
    partition_name = nc.partition_id_tensor.name if nc.partition_id_tensor else None
    in_names, out_names, out_avals, zero_outs = [], [], [], []
    for alloc in nc.m.functions[0].allocations:
        if not isinstance(alloc, _mb.MemoryLocationSet):
            continue
        name = alloc.memorylocations[0].name
        if alloc.kind == "ExternalInput":
            if name != partition_name:
                in_names.append(name)
        elif alloc.kind == "ExternalOutput":
            shape = tuple(alloc.tensor_shape)
            dtype = _mb.dt.np(alloc.dtype)
            out_names.append(name)
            out_avals.append(jax.core.ShapedArray(shape, dtype))
            zero_outs.append(np.zeros(shape, dtype))
    n_params = len(in_names)
    all_in = list(in_names) + list(out_names)
    if partition_name is not None:
        all_in.append(partition_name)

    def _body(*args):
        operands = list(args)
        if partition_name is not None:
            operands.append(b2j.partition_id_tensor())
        outs = b2j._bass_exec_p.bind(
            *operands, out_avals=tuple(out_avals), in_names=tuple(all_in),
            out_names=tuple(out_names), lowering_input_output_aliases=(),
            sim_require_finite=True, sim_require_nnan=True, nc=nc)
        return tuple(outs)

    cached = _PJRT_CACHE.get(id(nc))
    if cached is None:
        from jax.sharding import NamedSharding
        import jax.numpy as jnp
        devices = jax.devices()[:n_cores]
        mesh = Mesh(np.array(devices), ("core",))
        n_outs = len(out_names)
        sharded = jax.jit(
            shard_map(_body, mesh=mesh,
                      in_specs=(PartitionSpec("core"),) * (n_params + n_outs),
                      out_specs=(PartitionSpec("core"),) * n_outs,
                      check_rep=False),
            keep_unused=True)
        # Materialize zero output buffers directly on-device (sharded), so
        # they are never shipped from host on dispatch.
        zshapes = [((n_cores * z.shape[0], *z.shape[1:]), z.dtype)
                   for z in zero_outs]
        mk = jax.jit(
            lambda: tuple(jnp.zeros(s, d) for s, d in zshapes),
            out_shardings=tuple(
                NamedSharding(mesh, PartitionSpec("core"))
                for _ in zshapes))
        zero_args = list(jax.block_until_ready(mk()))
        _PJRT_CACHE[id(nc)] = (sharded, zero_args)
    else:
        sharded, zero_args = cached
    concat_in = [np.concatenate([np.asarray(in_maps[c][nm])
                                 for c in range(n_cores)], axis=0)
                 for nm in in_names]
    args = [jax.device_put(a) for a in concat_in] + zero_args
    out_arrs = jax.block_until_ready(sharded(*args))
    best = None
    all_ns = []
    for _ in range(time_iters):
        t0 = _time.perf_counter()
        out_arrs2 = jax.block_until_ready(sharded(*args))
        dt = _time.perf_counter() - t0
        all_ns.append(int(dt * 1e9))
        best = dt if best is None or dt < best else best
    kernel._all_ns = all_ns
    results = [{nm: np.asarray(out_arrs[i]).reshape(n_cores,
                                                    *out_avals[i].shape)[c]
                for i, nm in enumerate(out_names)}
               for c in range(n_cores)]
    return results, (None if best is None else int(best * 1e9))


def kernel(u_sequence, w_ih_f, w_hh_f, b_ih_f, b_hh_f,
           w_ih_b, w_hh_b, b_ih_b, b_hh_b, _time_iters=0):
    u = np.asarray(u_sequence, np.float32)
    Bn, T, Dn = u.shape
    assert (Bn, Dn) == (B, D)

    whh_i, wih_i, bias, rs = _prep_dir(w_ih_b, w_hh_b, b_ih_b, b_hh_b)
    ident = np.eye(128, dtype=_BF)
    s16 = np.zeros((16, 128), dtype=_BF)
    for q in range(4):
        for b_ in range(BL):
            s16[4 * q + b_, 32 * q + b_] = 1
    sel16 = np.ascontiguousarray(s16.T)
    aux = np.zeros((4, 256), dtype=_BF)
    aux[0, 128:256] = 1                       # ones row for xh broadcast
    for q in range(4):
        for b_ in range(BL):
            aux[q, 32 * q + b_] = 1           # block-diag selector mask

    unroll = 16
    use_loop = (T % unroll == 0) and T >= 16 \
        and not os.environ.get('KERNEL_NO_LOOP')
    nc = _get_program(T, unroll, use_loop)

    common = dict(whh_b=whh_i, wih_b=wih_i, bias_b=bias, rs_b=rs,
                  ident=ident, s16=s16, sel16=sel16, aux=aux)
    in_maps = []
    for c in range(N_CORES):
        m = dict(common)
        m["u"] = np.ascontiguousarray(u[c * BL:(c + 1) * BL])
        in_maps.append(m)

    results, best_ns = _run_pjrt(nc, in_maps, time_iters=_time_iters)
    out = np.concatenate([results[c]["out"] for c in range(N_CORES)], axis=0)
    kernel._last_ns = best_ns
    return out


if __name__ == "__main__":
    # tiny smoke: T=16 vs jax reference
    import reference
    T_s = 16
    inputs = reference.setup_inputs()
    inputs = {k: np.asarray(v) for k, v in inputs.items()}
    inputs["u_sequence"] = inputs["u_sequence"][:, :T_s, :]
    want = np.asarray(reference.reference(**inputs))
    got = kernel(**inputs)
    err = np.abs(got - want)
    rel = err.max() / np.abs(want).max()
    print(f"T={T_s}  maxabs={err.max():.3e}  rel={rel:.3e}")
